# revision 1
# baseline (speedup 1.0000x reference)
"""GCNII encoder + KNN label-fusion subgraph on 8 Trainium2 NeuronCores.

Sharding: nodes (rows) split into 8 blocks of N/8. Each core:
  - computes h = relu(x_blk @ W_in + b_in)
  - 9 GCNII layers: agg_blk = A[blk, :] @ h_full  (dense fp16 adjacency,
    streamed from HBM as PE matmuls), h_full re-AllGathered (fp16) per layer
  - p_lc = log_softmax(emb @ W_out + b_out) on its rows
  - cosine-sim branch: en = emb/||emb||; per-row exact top-16 threshold tau
    via max8/match_replace8 over PSUM sim strips; fused = (exp(sim) *
    (sim >= tau)) @ one_hot(y) as PE matmuls; p_sim = log_softmax(fused)
  - out = 0.5*p_lc + 0.5*p_sim
Host only preps layouts: dense A^T blocks (fp16), transposed x, one-hot(y).
"""
import math
from contextlib import ExitStack

import numpy as np

import concourse.bass as bass
import concourse.tile as tile
from concourse import bacc, mybir
from concourse.bass_utils import run_bass_kernel_spmd
from concourse.masks import make_identity

F32 = mybir.dt.float32
F16 = mybir.dt.float16
AF = mybir.ActivationFunctionType
ALU = mybir.AluOpType

N_CORES = 8
N = 16384
D_IN = 512
H = 256
C = 64
K_TOP = 16
N_LAYERS = 9
ALPHA = 0.5
THETA = 1.0
NEG = -1e30


def _betas():
    return [float(np.log(THETA / (l + 1) + 1.0)) for l in range(N_LAYERS)]


def build_program(n=N, n_layers=N_LAYERS):
    blk = n // N_CORES          # rows per core
    n_it = blk // 128           # 128-row tiles per block
    igw = min(512, blk)         # i-group width (dst cols per psum tile)
    n_ig = blk // igw
    n_js = n // 128             # src slabs
    chunkw = min(1024, n)       # S1 scan chunk width
    n_chunk = n // chunkw
    subw = min(512, blk)        # sim rhs tile width (<= c-block, <= 512)
    betas = _betas()

    nc = bacc.Bacc("TRN2", target_bir_lowering=False, debug=False,
                   num_devices=N_CORES)

    xT_d = nc.dram_tensor("xT", [D_IN, blk], F32, kind="ExternalInput")
    at16_d = nc.dram_tensor("at16", [n_js, n_ig, 128, igw], F16,
                            kind="ExternalInput")
    w_in_d = nc.dram_tensor("w_in", [D_IN, H], F32, kind="ExternalInput")
    b_in_d = nc.dram_tensor("b_in_r", [1, H], F32, kind="ExternalInput")
    cw1_d = nc.dram_tensor("cw1", [n_layers, H, H], F32, kind="ExternalInput")
    cw2_d = nc.dram_tensor("cw2", [n_layers, H, H], F32, kind="ExternalInput")
    w_out_d = nc.dram_tensor("w_out", [H, C], F32, kind="ExternalInput")
    b_out_d = nc.dram_tensor("b_out_r", [1, C], F32, kind="ExternalInput")
    oh_d = nc.dram_tensor("oh16", [n, C], F16, kind="ExternalInput")
    out_d = nc.dram_tensor("out", [blk, C], F32, kind="ExternalOutput")

    groups = [list(range(N_CORES))]

    with tile.TileContext(nc) as tc, ExitStack() as S:
        const = S.enter_context(tc.tile_pool(name="const", bufs=1))
        dram = S.enter_context(tc.tile_pool(name="dram", bufs=1, space="DRAM"))
        hT_pool = S.enter_context(tc.tile_pool(name="hTp", bufs=2))
        # GCN-phase pools, released before the similarity phase
        G = ExitStack()
        x0pool = G.enter_context(tc.tile_pool(name="x0p", bufs=1))
        hfull_pool = G.enter_context(tc.tile_pool(name="hfp", bufs=1))
        h16b_pool = G.enter_context(tc.tile_pool(name="h16bp", bufs=2))

        ident = const.tile([128, 128], F32)
        make_identity(nc, ident[:])
        ident16 = const.tile([128, 128], F16)
        nc.vector.tensor_copy(ident16[:], ident[:])
        ones1 = const.tile([1, 128], F32)
        nc.vector.memset(ones1[:], 1.0)
        w_in_sb = const.tile([128, D_IN // 128, H], F32)
        nc.sync.dma_start(w_in_sb[:], w_in_d.ap().rearrange("(k p) d -> p k d", p=128))
        b_in_sb = const.tile([1, H], F32)
        nc.sync.dma_start(b_in_sb[:], b_in_d.ap())
        w_out_sb = const.tile([128, 2, C], F32)
        nc.sync.dma_start(w_out_sb[:], w_out_d.ap().rearrange("(k p) c -> p k c", p=128))
        b_out_sb = const.tile([1, C], F32)
        nc.sync.dma_start(b_out_sb[:], b_out_d.ap())
        oh_sb = const.tile([128, n_js, C], F16)
        nc.sync.dma_start(oh_sb[:], oh_d.ap().rearrange("(s p) c -> p s c", p=128))

        x0sT = x0pool.tile([128, 2, blk], F32)
        out_acc = const.tile([128, n_it, C], F32)

        def logsoftmax_from_psum(dst_ap, psrc, sp, add_into=None):
            """dst = 0.5 * log_softmax(psrc rows); psrc is [128, C] psum."""
            m = sp.tile([128, 1], F32, tag="ls_m")
            nc.vector.reduce_max(out=m[:], in_=psrc[:], axis=mybir.AxisListType.X)
            mneg = sp.tile([128, 1], F32, tag="ls_mn")
            nc.vector.tensor_scalar_mul(mneg[:], m[:], -1.0)
            e = sp.tile([128, C], F32, tag="ls_e")
            ssum = sp.tile([128, 1], F32, tag="ls_s")
            nc.scalar.activation(e[:], psrc[:], AF.Exp, bias=mneg[:], scale=1.0,
                                 accum_out=ssum[:])
            ls = sp.tile([128, 1], F32, tag="ls_l")
            nc.scalar.activation(ls[:], ssum[:], AF.Ln)
            m2 = sp.tile([128, 1], F32, tag="ls_m2")
            nc.vector.tensor_add(m2[:], m[:], ls[:])
            if add_into is None:
                nc.vector.tensor_scalar(dst_ap, psrc[:], m2[:], 0.5,
                                        op0=ALU.subtract, op1=ALU.mult)
            else:
                t = sp.tile([128, C], F32, tag="ls_t")
                nc.vector.tensor_scalar(t[:], psrc[:], m2[:], 0.5,
                                        op0=ALU.subtract, op1=ALU.mult)
                nc.vector.tensor_add(dst_ap, add_into, t[:])

        def allgather_h16(h16_blk_t, tag):
            gin = dram.tile([128, n_it, H], F16, tag=f"{tag}_in")
            nc.sync.dma_start(gin[:], h16_blk_t[:])
            gout = dram.tile([N_CORES, 128, n_it, H], F16, tag=f"{tag}_out",
                             addr_space="Shared")
            nc.gpsimd.collective_compute(
                "AllGather", ALU.bypass, replica_groups=groups,
                ins=[gin[:].opt()], outs=[gout[:].opt()])
            hf = hfull_pool.tile([128, N_CORES, n_it, H], F16, tag="hfull")
            nc.sync.dma_start(hf[:], gout[:].rearrange("c p s d -> p c s d"))
            return hf

        # ---------- phase 0: h0 = relu(x @ W_in + b_in) ----------
        with (
            tc.tile_pool(name="p0ps", bufs=2, space="PSUM") as p0ps,
            tc.tile_pool(name="p0sb", bufs=3) as p0sb,
            tc.tile_pool(name="p0x", bufs=1) as p0x,
        ):
            xT_sb = p0x.tile([128, D_IN // 128, blk], F32)
            nc.sync.dma_start(xT_sb[:], xT_d.ap().rearrange("(k p) i -> p k i", p=128))
            hT = hT_pool.tile([128, 2, blk], F32, tag="hT")
            h16_blk = h16b_pool.tile([128, n_it, H], F16, tag="h16b")
            for it in range(n_it):
                ph = p0ps.tile([128, H], F32, tag="ph")
                for k in range(D_IN // 128):
                    nc.tensor.matmul(ph[:], xT_sb[:, k, it * 128:(it + 1) * 128],
                                     w_in_sb[:, k, :], start=(k == 0), stop=False)
                nc.tensor.matmul(ph[:], ones1[:], b_in_sb[:], start=False, stop=True)
                hm = p0sb.tile([128, H], F32, tag="hm")
                nc.scalar.activation(hm[:], ph[:], AF.Relu)
                nc.vector.tensor_copy(h16_blk[:, it, :], hm[:])
                for dh in range(2):
                    pt = p0ps.tile([128, 128], F32, tag="pt")
                    nc.tensor.transpose(pt[:], hm[:, dh * 128:(dh + 1) * 128], ident[:])
                    nc.scalar.activation(hT[:, dh, it * 128:(it + 1) * 128], pt[:], AF.Copy)
            nc.vector.tensor_scalar_mul(x0sT[:], hT[:], 0.5)
        h16_full = allgather_h16(h16_blk, "ag")

        # ---------- GCN layers ----------
        with (
            tc.tile_pool(name="aggps", bufs=2, space="PSUM") as aggps,
            tc.tile_pool(name="mmps", bufs=2, space="PSUM") as mmps,
            tc.tile_pool(name="tps", bufs=2, space="PSUM") as tps,
            tc.tile_pool(name="apool", bufs=6) as apool,
            tc.tile_pool(name="wpool", bufs=2) as wpool,
            tc.tile_pool(name="xpool", bufs=2) as xpool,
            tc.tile_pool(name="tpool", bufs=3) as tpool,
        ):
            for l in range(n_layers):
                beta = betas[l]
                cw1_sb = wpool.tile([128, 2, H], F32, tag="cw1")
                nc.sync.dma_start(cw1_sb[:],
                                  cw1_d.ap()[l].rearrange("(k p) d -> p k d", p=128))
                cw2_sb = wpool.tile([128, 2, H], F32, tag="cw2")
                nc.sync.dma_start(cw2_sb[:],
                                  cw2_d.ap()[l].rearrange("(k p) d -> p k d", p=128))
                hT_new = hT_pool.tile([128, 2, blk], F32, tag="hT")
                for ig in range(n_ig):
                    pa0 = aggps.tile([128, igw], F32, tag="agg0")
                    pa1 = aggps.tile([128, igw], F32, tag="agg1")
                    for js in range(n_js):
                        a_t = apool.tile([128, igw], F16, tag="a")
                        nc.sync.dma_start(a_t[:], at16_d.ap()[js, ig])
                        jc, jb = divmod(js, n_it)
                        nc.tensor.matmul(pa0[:], h16_full[:, jc, jb, 0:128], a_t[:],
                                         start=(js == 0), stop=(js == n_js - 1))
                        nc.tensor.matmul(pa1[:], h16_full[:, jc, jb, 128:256], a_t[:],
                                         start=(js == 0), stop=(js == n_js - 1))
                    xsT = xpool.tile([128, 2, igw], F32, tag="xsT")
                    nc.scalar.activation(xsT[:, 0, :], pa0[:], AF.Copy, scale=0.5)
                    nc.scalar.activation(xsT[:, 1, :], pa1[:], AF.Copy, scale=0.5)
                    sl = slice(ig * igw, (ig + 1) * igw)
                    for dh in range(2):
                        pmm = mmps.tile([128, igw], F32, tag="pmm")
                        nc.tensor.matmul(pmm[:], cw1_sb[:, 0, dh * 128:(dh + 1) * 128],
                                         xsT[:, 0, :], start=True, stop=False)
                        nc.tensor.matmul(pmm[:], cw1_sb[:, 1, dh * 128:(dh + 1) * 128],
                                         xsT[:, 1, :], start=False, stop=False)
                        nc.tensor.matmul(pmm[:], cw2_sb[:, 0, dh * 128:(dh + 1) * 128],
                                         x0sT[:, 0, sl], start=False, stop=False)
                        nc.tensor.matmul(pmm[:], cw2_sb[:, 1, dh * 128:(dh + 1) * 128],
                                         x0sT[:, 1, sl], start=False, stop=True)
                        t1 = tpool.tile([128, igw], F32, tag="t1")
                        nc.vector.tensor_add(t1[:], xsT[:, dh, :], x0sT[:, dh, sl])
                        t2 = tpool.tile([128, igw], F32, tag="t2")
                        nc.scalar.activation(t2[:], pmm[:], AF.Copy, scale=beta)
                        nc.vector.tensor_scalar_mul(t1[:], t1[:], 1.0 - beta)
                        nc.vector.tensor_add(t1[:], t1[:], t2[:])
                        nc.vector.tensor_add(t1[:], t1[:], hT[:, dh, sl])
                        nc.scalar.activation(hT_new[:, dh, sl], t1[:], AF.Relu)
                hT = hT_new
                if l < n_layers - 1:
                    h16_new = h16b_pool.tile([128, n_it, H], F16, tag="h16b")
                    for it in range(n_it):
                        for dh in range(2):
                            pt = tps.tile([128, 128], F32, tag="pt")
                            nc.tensor.transpose(
                                pt[:], hT[:, dh, it * 128:(it + 1) * 128], ident[:])
                            nc.scalar.activation(
                                h16_new[:, it, dh * 128:(dh + 1) * 128], pt[:], AF.Copy)
                    h16_full = allgather_h16(h16_new, "ag")
        embT = hT  # [128, 2, blk] f32
        G.close()  # release GCN-phase SBUF (h16_full, x0sT, h16_blk)
        spool = S.enter_context(tc.tile_pool(name="spool", bufs=1))

        # ---------- p_lc ----------
        with (
            tc.tile_pool(name="lcps", bufs=2, space="PSUM") as lcps,
            tc.tile_pool(name="lcsb", bufs=2) as lcsb,
        ):
            for it in range(n_it):
                plc = lcps.tile([128, C], F32, tag="plc")
                nc.tensor.matmul(plc[:], embT[:, 0, it * 128:(it + 1) * 128],
                                 w_out_sb[:, 0, :], start=True, stop=False)
                nc.tensor.matmul(plc[:], embT[:, 1, it * 128:(it + 1) * 128],
                                 w_out_sb[:, 1, :], start=False, stop=False)
                nc.tensor.matmul(plc[:], ones1[:], b_out_sb[:], start=False, stop=True)
                logsoftmax_from_psum(out_acc[:, it, :], plc, lcsb)

        # ---------- normalize ----------
        enT16_blk = spool.tile([128, 2, blk], F16)
        with (
            tc.tile_pool(name="nps", bufs=2, space="PSUM") as nps,
            tc.tile_pool(name="nsb", bufs=3) as nsb,
        ):
            en16_blk = nsb.tile([128, n_it, H], F16, tag="en16b", bufs=1)
            for it in range(n_it):
                pn0 = nps.tile([128, 128], F32, tag="pn0")
                nc.tensor.transpose(pn0[:], embT[:, 0, it * 128:(it + 1) * 128], ident[:])
                pn1 = nps.tile([128, 128], F32, tag="pn1")
                nc.tensor.transpose(pn1[:], embT[:, 1, it * 128:(it + 1) * 128], ident[:])
                emb_n = nsb.tile([128, H], F32, tag="embn")
                nc.scalar.activation(emb_n[:, 0:128], pn0[:], AF.Copy)
                nc.scalar.activation(emb_n[:, 128:256], pn1[:], AF.Copy)
                sq = nsb.tile([128, H], F32, tag="sq")
                ss = nsb.tile([128, 1], F32, tag="ss")
                nc.scalar.activation(sq[:], emb_n[:], AF.Square, accum_out=ss[:])
                nrm = nsb.tile([128, 1], F32, tag="nrm")
                nc.scalar.activation(nrm[:], ss[:], AF.Sqrt)
                nc.vector.tensor_scalar_max(nrm[:], nrm[:], 1e-8)
                inv = nsb.tile([128, 1], F32, tag="inv")
                nc.vector.reciprocal(inv[:], nrm[:])
                nc.vector.tensor_scalar(en16_blk[:, it, :], emb_n[:], inv[:], None,
                                        op0=ALU.mult)
                for dh in range(2):
                    pt = nps.tile([128, 128], F16, tag="pt2")
                    nc.tensor.transpose(
                        pt[:], en16_blk[:, it, dh * 128:(dh + 1) * 128], ident16[:])
                    nc.scalar.activation(
                        enT16_blk[:, dh, it * 128:(it + 1) * 128], pt[:], AF.Copy)
            gin2 = dram.tile([128, 2, blk], F16, tag="eg_in")
            nc.sync.dma_start(gin2[:], enT16_blk[:])
            gout2 = dram.tile([N_CORES, 128, 2, blk], F16, tag="eg_out",
                              addr_space="Shared")
            nc.gpsimd.collective_compute(
                "AllGather", ALU.bypass, replica_groups=groups,
                ins=[gin2[:].opt()], outs=[gout2[:].opt()])
            enT16_full = spool.tile([128, 2, N_CORES, blk], F16)
            nc.sync.dma_start(enT16_full[:], gout2[:].rearrange("c p h i -> p h c i"))

        # ---------- S1: per-row top-16 threshold tau ----------
        tau_rep = spool.tile([128, blk], F32)
        with (
            tc.tile_pool(name="sps", bufs=2, space="PSUM") as sps,
            tc.tile_pool(name="t8ps", bufs=2, space="PSUM") as t8ps,
            tc.tile_pool(name="s1sb", bufs=2) as s1sb,
        ):
            tau_col = s1sb.tile([128, n_it], F32, tag="tau_col", bufs=1)
            for it in range(n_it):
                cands = s1sb.tile([128, n_chunk * 16], F32, tag="cands")
                for ch in range(n_chunk):
                    strip = sps.tile([128, chunkw], F32, tag="strip")
                    for st in range(chunkw // subw):
                        j0 = ch * chunkw + st * subw
                        cb, off = divmod(j0, blk)
                        nc.tensor.matmul(
                            strip[:, st * subw:(st + 1) * subw],
                            enT16_blk[:, 0, it * 128:(it + 1) * 128],
                            enT16_full[:, 0, cb, off:off + subw],
                            start=True, stop=False)
                        nc.tensor.matmul(
                            strip[:, st * subw:(st + 1) * subw],
                            enT16_blk[:, 1, it * 128:(it + 1) * 128],
                            enT16_full[:, 1, cb, off:off + subw],
                            start=False, stop=True)
                    nc.vector.max(out=cands[:, ch * 16:ch * 16 + 8], in_=strip[:])
                    nc.vector.match_replace(out=strip[:],
                                            in_to_replace=cands[:, ch * 16:ch * 16 + 8],
                                            in_values=strip[:], imm_value=NEG)
                    nc.vector.max(out=cands[:, ch * 16 + 8:ch * 16 + 16], in_=strip[:])
                m1 = s1sb.tile([128, 8], F32, tag="m1")
                nc.vector.max(out=m1[:], in_=cands[:])
                nc.vector.match_replace(out=cands[:], in_to_replace=m1[:],
                                        in_values=cands[:], imm_value=NEG)
                m2 = s1sb.tile([128, 8], F32, tag="m2")
                nc.vector.max(out=m2[:], in_=cands[:])
                nc.vector.tensor_copy(tau_col[:, it:it + 1], m2[:, 7:8])
            # tau_col [128, n_it] -> tauT [n_it, 128] -> row [1, blk] -> tau_rep
            ptt = t8ps.tile([128, 128], F32, tag="ptt")
            nc.tensor.transpose(ptt[:n_it, :], tau_col[:], ident[:])
            tauT = s1sb.tile([n_it, 128], F32, tag="tauT", bufs=1)
            nc.scalar.activation(tauT[:], ptt[:n_it, :], AF.Copy)
            taurow = s1sb.tile([1, blk], F32, tag="taurow", bufs=1)
            nc.sync.dma_start(taurow[:], tauT[:])
            bw = min(512, blk)
            for bb in range(blk // bw):
                pb = t8ps.tile([128, bw], F32, tag="pb")
                nc.tensor.matmul(pb[:], ones1[:], taurow[:, bb * bw:(bb + 1) * bw],
                                 start=True, stop=True)
                nc.scalar.activation(tau_rep[:, bb * bw:(bb + 1) * bw], pb[:], AF.Copy)

        # ---------- S2: fused = (exp(sim) * (sim >= tau)) @ OH; p_sim ----------
        with (
            tc.tile_pool(name="simps", bufs=3, space="PSUM") as simps,
            tc.tile_pool(name="fps", bufs=2, space="PSUM") as fps,
            tc.tile_pool(name="ftps", bufs=2, space="PSUM") as ftps,
            tc.tile_pool(name="s2sb", bufs=3) as s2sb,
        ):
            for ig in range(n_ig):
                sl = slice(ig * igw, (ig + 1) * igw)
                pfused = fps.tile([C, igw], F32, tag="pf")
                for jt in range(n_js):
                    cb, off = divmod(jt * 128, blk)
                    psim = simps.tile([128, igw], F32, tag="psim")
                    nc.tensor.matmul(psim[:], enT16_full[:, 0, cb, off:off + 128],
                                     enT16_blk[:, 0, sl], start=True, stop=False)
                    nc.tensor.matmul(psim[:], enT16_full[:, 1, cb, off:off + 128],
                                     enT16_blk[:, 1, sl], start=False, stop=True)
                    e16 = s2sb.tile([128, igw], F16, tag="e16")
                    nc.scalar.activation(e16[:], psim[:], AF.Exp)
                    mk16 = s2sb.tile([128, igw], F16, tag="mk16")
                    nc.vector.tensor_tensor(mk16[:], psim[:], tau_rep[:, sl],
                                            op=ALU.is_ge)
                    ew16 = s2sb.tile([128, igw], F16, tag="ew16")
                    nc.vector.tensor_mul(ew16[:], e16[:], mk16[:])
                    nc.tensor.matmul(pfused[:], oh_sb[:, jt, :], ew16[:],
                                     start=(jt == 0), stop=(jt == n_js - 1))
                fsb = s2sb.tile([C, igw], F32, tag="fsb")
                nc.scalar.activation(fsb[:], pfused[:], AF.Copy)
                for t in range(igw // 128):
                    it = ig * (igw // 128) + t
                    pft = ftps.tile([128, C], F32, tag="pft")
                    nc.tensor.transpose(pft[:, :C], fsb[:, t * 128:(t + 1) * 128],
                                        ident[:C, :C])
                    logsoftmax_from_psum(out_acc[:, it, :], pft[:, :C], s2sb,
                                         add_into=out_acc[:, it, :])
            nc.sync.dma_start(out_d.ap().rearrange("(it p) c -> p it c", p=128),
                              out_acc[:])

    nc.compile()
    return nc


def prep_inputs(inputs, n=N, n_layers=N_LAYERS):
    """Host-side sharding/layout prep. Returns in_maps (one dict per core)."""
    blk = n // N_CORES
    igw = min(512, blk)
    n_ig = blk // igw
    n_js = n // 128
    x = np.asarray(inputs["x"], np.float32)
    y = np.asarray(inputs["y"])
    ei = np.asarray(inputs["edge_index"])
    ew = np.asarray(inputs["edge_weight"], np.float32)
    src, dst = ei[0].astype(np.int64), ei[1].astype(np.int64)

    oh = np.zeros((n, C), np.float16)
    oh[np.arange(n), y.astype(np.int64)] = 1.0
    w_in = np.ascontiguousarray(np.asarray(inputs["W_in"], np.float32))
    b_in = np.asarray(inputs["b_in"], np.float32).reshape(1, H)
    w_out = np.ascontiguousarray(np.asarray(inputs["W_out"], np.float32))
    b_out = np.asarray(inputs["b_out"], np.float32).reshape(1, C)
    cw1 = np.ascontiguousarray(np.asarray(inputs["conv_w1"], np.float32))
    cw2 = np.ascontiguousarray(np.asarray(inputs["conv_w2"], np.float32))

    in_maps = []
    for c in range(N_CORES):
        lo, hi = c * blk, (c + 1) * blk
        sel = (dst >= lo) & (dst < hi)
        # AT_c[src, dst_local] = sum of w over duplicate edges
        flat = src[sel] * blk + (dst[sel] - lo)
        at = np.bincount(flat, weights=ew[sel].astype(np.float64),
                         minlength=n * blk).astype(np.float32).reshape(n, blk)
        at16 = np.ascontiguousarray(
            at.reshape(n_js, 128, n_ig, igw).transpose(0, 2, 1, 3)).astype(np.float16)
        xT = np.ascontiguousarray(x[lo:hi].T)
        in_maps.append({
            "xT": xT, "at16": at16, "w_in": w_in, "b_in_r": b_in,
            "cw1": cw1, "cw2": cw2, "w_out": w_out, "b_out_r": b_out,
            "oh16": oh,
        })
    return in_maps


_CACHED_NC = None


def kernel(**inputs):
    global _CACHED_NC
    if _CACHED_NC is None:
        _CACHED_NC = build_program()
    in_maps = prep_inputs(inputs)
    res = run_bass_kernel_spmd(_CACHED_NC, in_maps, core_ids=list(range(N_CORES)))
    out = np.concatenate([res.results[c]["out"] for c in range(N_CORES)], axis=0)
    return out.astype(np.float32)


if __name__ == "__main__":
    nc = build_program()
    print("built + compiled OK")



# revision 3
# speedup vs baseline: 133.8026x; 133.8026x over previous
"""GCNII encoder + KNN label-fusion subgraph on 8 Trainium2 NeuronCores.

Sharding: nodes (rows) split into 8 blocks of N/8. Each core:
  - builds its dense fp16 adjacency block A^T[:, blk] ON DEVICE from a
    compact padded COO edge list (iota-compare one-hots + PE matmuls),
    so only ~6 MiB/core of inputs cross the host link instead of 512 MiB
  - computes h = relu(x_blk @ W_in + b_in)
  - 9 GCNII layers: agg_blk = A[blk, :] @ h_full (dense fp16 adjacency
    streamed from device HBM as PE matmuls), h_full re-AllGathered (fp16)
  - p_lc = log_softmax(emb @ W_out + b_out) on its rows
  - cosine-sim branch: en = emb/||emb||; per-row exact top-16 threshold tau
    via max8/match_replace8 over PSUM sim strips; fused = (exp(sim) *
    (sim >= tau)) @ one_hot(y) as PE matmuls; p_sim = log_softmax(fused)
  - out = 0.5*p_lc + 0.5*p_sim
Host only preps layouts: bucketed edge lists, transposed x, weights.
"""
import math
from contextlib import ExitStack

import numpy as np

import concourse.bass as bass
import concourse.tile as tile
from concourse import bacc, mybir
from concourse.bass_utils import run_bass_kernel_spmd
from concourse.masks import make_identity

F32 = mybir.dt.float32
F16 = mybir.dt.float16
I32 = mybir.dt.int32
AF = mybir.ActivationFunctionType
ALU = mybir.AluOpType

N_CORES = 8
N = 16384
D_IN = 512
H = 256
C = 64
K_TOP = 16
N_LAYERS = 9
ALPHA = 0.5
THETA = 1.0
NEG = -1e30

# edge bucketing: bucket = (src slab js, dst window ig), capacity CAP edges
CAP = 256            # >= 11 sigma above Poisson(128) mean; overflow ~impossible
N_CH = CAP // 128    # 128-entry chunks per bucket


def _betas():
    return [float(np.log(THETA / (l + 1) + 1.0)) for l in range(N_LAYERS)]


def build_program(n=N, n_layers=N_LAYERS):
    blk = n // N_CORES          # rows per core
    n_it = blk // 128           # 128-row tiles per block
    igw = min(512, blk)         # i-group width (dst cols per psum tile)
    n_ig = blk // igw
    n_js = n // 128             # src slabs
    chunkw = min(1024, n)       # S1 scan chunk width
    n_chunk = n // chunkw
    subw = min(512, blk)        # sim rhs tile width (<= c-block, <= 512)
    betas = _betas()

    nc = bacc.Bacc("TRN2", target_bir_lowering=False, debug=False,
                   num_devices=N_CORES)

    xT_d = nc.dram_tensor("xT16", [128, D_IN // 128, blk], F16,
                          kind="ExternalInput")
    esrc_d = nc.dram_tensor("esrc", [128, n_js, n_ig, N_CH], F32,
                            kind="ExternalInput")
    edst_d = nc.dram_tensor("edst", [128, n_js, n_ig, N_CH], F32,
                            kind="ExternalInput")
    ew_d = nc.dram_tensor("ew", [128, n_js, n_ig, N_CH], F32,
                          kind="ExternalInput")
    y_d = nc.dram_tensor("y_r", [128, n_js], F32, kind="ExternalInput")
    w_in_d = nc.dram_tensor("w_in16", [128, D_IN // 128, H], F16,
                            kind="ExternalInput")
    b_in_d = nc.dram_tensor("b_in16", [1, H], F16, kind="ExternalInput")
    cw1_d = nc.dram_tensor("cw116", [n_layers, 128, 2, H], F16,
                           kind="ExternalInput")
    cw2_d = nc.dram_tensor("cw216", [n_layers, 128, 2, H], F16,
                           kind="ExternalInput")
    w_out_d = nc.dram_tensor("w_out_r", [128, 2, C], F32, kind="ExternalInput")
    b_out_d = nc.dram_tensor("b_out_r", [1, C], F32, kind="ExternalInput")
    out_d = nc.dram_tensor("out", [blk, C], F32, kind="ExternalOutput")

    groups = [list(range(N_CORES))]

    with tile.TileContext(nc) as tc, ExitStack() as S:
        const = S.enter_context(tc.tile_pool(name="const", bufs=1))
        dram = S.enter_context(tc.tile_pool(name="dram", bufs=1, space="DRAM"))
        hT_pool = S.enter_context(tc.tile_pool(name="hTp", bufs=2))
        # GCN-phase pools, released before the similarity phase
        G = ExitStack()
        x0pool = G.enter_context(tc.tile_pool(name="x0p", bufs=1))
        hfull_pool = G.enter_context(tc.tile_pool(name="hfp", bufs=1))
        h16b_pool = G.enter_context(tc.tile_pool(name="h16bp", bufs=2))

        ident = const.tile([128, 128], F32)
        make_identity(nc, ident[:])
        ident16 = const.tile([128, 128], F16)
        nc.vector.tensor_copy(ident16[:], ident[:])
        ones1 = const.tile([1, 128], F32)
        nc.vector.memset(ones1[:], 1.0)
        ones1_16 = const.tile([1, 128], F16)
        nc.vector.memset(ones1_16[:], 1.0)
        # iota tiles for one-hot construction (values exact in f16)
        iota_w_i = const.tile([128, igw], I32)
        nc.gpsimd.iota(iota_w_i[:], pattern=[[1, igw]], channel_multiplier=0)
        iota_w = const.tile([128, igw], F32)
        nc.vector.tensor_copy(iota_w[:], iota_w_i[:])
        iota_p_i = const.tile([128, 128], I32)
        nc.gpsimd.iota(iota_p_i[:], pattern=[[1, 128]], channel_multiplier=0)
        iota_p = const.tile([128, 128], F32)
        nc.vector.tensor_copy(iota_p[:], iota_p_i[:])
        iota_c = const.tile([128, C], F32)
        nc.vector.tensor_copy(iota_c[:], iota_p_i[:, :C])

        w_in_sb = const.tile([128, D_IN // 128, H], F16)
        nc.sync.dma_start(w_in_sb[:], w_in_d.ap())
        b_in_sb = const.tile([1, H], F16)
        nc.sync.dma_start(b_in_sb[:], b_in_d.ap())
        w_out_sb = const.tile([128, 2, C], F32)
        nc.sync.dma_start(w_out_sb[:], w_out_d.ap())
        b_out_sb = const.tile([1, C], F32)
        nc.sync.dma_start(b_out_sb[:], b_out_d.ap())

        # ---------- one-hot(y) on device ----------
        y_sb = const.tile([128, n_js], F32)
        nc.sync.dma_start(y_sb[:], y_d.ap())
        oh_sb = const.tile([128, n_js, C], F16)
        for js in range(n_js):
            nc.vector.tensor_scalar(oh_sb[:, js, :], iota_c[:],
                                    y_sb[:, js:js + 1], None, op0=ALU.is_equal)

        x0sT = x0pool.tile([128, 2, blk], F32)
        x0sT16 = x0pool.tile([128, 2, blk], F16)
        out_acc = const.tile([128, n_it, C], F32)

        def logsoftmax_from_psum(dst_ap, psrc, sp, add_into=None):
            """dst = 0.5 * log_softmax(psrc rows); psrc is [128, C] psum."""
            m = sp.tile([128, 1], F32, tag="ls_m")
            nc.vector.reduce_max(out=m[:], in_=psrc[:], axis=mybir.AxisListType.X)
            mneg = sp.tile([128, 1], F32, tag="ls_mn")
            nc.vector.tensor_scalar_mul(mneg[:], m[:], -1.0)
            e = sp.tile([128, C], F32, tag="ls_e")
            ssum = sp.tile([128, 1], F32, tag="ls_s")
            nc.scalar.activation(e[:], psrc[:], AF.Exp, bias=mneg[:], scale=1.0,
                                 accum_out=ssum[:])
            ls = sp.tile([128, 1], F32, tag="ls_l")
            nc.scalar.activation(ls[:], ssum[:], AF.Ln)
            m2 = sp.tile([128, 1], F32, tag="ls_m2")
            nc.vector.tensor_add(m2[:], m[:], ls[:])
            if add_into is None:
                nc.vector.tensor_scalar(dst_ap, psrc[:], m2[:], 0.5,
                                        op0=ALU.subtract, op1=ALU.mult)
            else:
                t = sp.tile([128, C], F32, tag="ls_t")
                nc.vector.tensor_scalar(t[:], psrc[:], m2[:], 0.5,
                                        op0=ALU.subtract, op1=ALU.mult)
                nc.vector.tensor_add(dst_ap, add_into, t[:])

        def allgather_h16(h16_blk_t, tag):
            gin = dram.tile([128, n_it, H], F16, tag=f"{tag}_in")
            nc.sync.dma_start(gin[:], h16_blk_t[:])
            gout = dram.tile([N_CORES, 128, n_it, H], F16, tag=f"{tag}_out",
                             addr_space="Shared")
            nc.gpsimd.collective_compute(
                "AllGather", ALU.bypass, replica_groups=groups,
                ins=[gin[:].opt()], outs=[gout[:].opt()])
            hf = hfull_pool.tile([128, N_CORES, n_it, H], F16, tag="hfull")
            nc.sync.dma_start(hf[:], gout[:].rearrange("c p s d -> p c s d"))
            return hf

        # ---------- phase 0: h0 = relu(x @ W_in + b_in) ----------
        with (
            tc.tile_pool(name="p0ps", bufs=2, space="PSUM") as p0ps,
            tc.tile_pool(name="p0sb", bufs=3) as p0sb,
            tc.tile_pool(name="p0x", bufs=1) as p0x,
        ):
            xT_sb = p0x.tile([128, D_IN // 128, blk], F16)
            nc.sync.dma_start(xT_sb[:], xT_d.ap())
            hT = hT_pool.tile([128, 2, blk], F32, tag="hT")
            h16_blk = h16b_pool.tile([128, n_it, H], F16, tag="h16b")
            for it in range(n_it):
                ph = p0ps.tile([128, H], F32, tag="ph")
                for k in range(D_IN // 128):
                    nc.tensor.matmul(ph[:], xT_sb[:, k, it * 128:(it + 1) * 128],
                                     w_in_sb[:, k, :], start=(k == 0), stop=False)
                nc.tensor.matmul(ph[:], ones1_16[:], b_in_sb[:], start=False,
                                 stop=True)
                hm = p0sb.tile([128, H], F32, tag="hm")
                nc.scalar.activation(hm[:], ph[:], AF.Relu)
                nc.vector.tensor_copy(h16_blk[:, it, :], hm[:])
                for dh in range(2):
                    pt = p0ps.tile([128, 128], F32, tag="pt")
                    nc.tensor.transpose(pt[:], hm[:, dh * 128:(dh + 1) * 128], ident[:])
                    nc.scalar.activation(hT[:, dh, it * 128:(it + 1) * 128], pt[:], AF.Copy)
            nc.vector.tensor_scalar_mul(x0sT[:], hT[:], 0.5)
            nc.vector.tensor_copy(x0sT16[:], x0sT[:])
        h16_full = allgather_h16(h16_blk, "ag")

        # ---------- build dense A^T on device from bucketed COO ----------
        # at_dram[js, ig, p, j] = sum over edges (src=js*128+p -> dst ig*igw+j)
        at_dram = dram.tile([n_js, n_ig, 128, igw], F16)
        with (
            tc.tile_pool(name="abps", bufs=2, space="PSUM") as abps,
            tc.tile_pool(name="absb", bufs=3) as absb,
            tc.tile_pool(name="aesb", bufs=1) as aesb,
        ):
            esrc_sb = aesb.tile([128, n_js, n_ig, N_CH], F32)
            nc.sync.dma_start(esrc_sb[:], esrc_d.ap())
            edst_sb = aesb.tile([128, n_js, n_ig, N_CH], F32)
            nc.sync.dma_start(edst_sb[:], edst_d.ap())
            ew_sb = aesb.tile([128, n_js, n_ig, N_CH], F32)
            nc.sync.dma_start(ew_sb[:], ew_d.ap())
            for ig in range(n_ig):
                for js in range(n_js):
                    pa = abps.tile([128, igw], F32, tag="pa")
                    for ch in range(N_CH):
                        ohs = absb.tile([128, 128], F16, tag="ohs")
                        nc.vector.tensor_scalar(
                            ohs[:], iota_p[:], esrc_sb[:, js, ig, ch:ch + 1],
                            None, op0=ALU.is_equal)
                        ohd = absb.tile([128, igw], F16, tag="ohd")
                        nc.vector.tensor_scalar(
                            ohd[:], iota_w[:], edst_sb[:, js, ig, ch:ch + 1],
                            ew_sb[:, js, ig, ch:ch + 1],
                            op0=ALU.is_equal, op1=ALU.mult)
                        nc.tensor.matmul(pa[:], ohs[:], ohd[:],
                                         start=(ch == 0), stop=(ch == N_CH - 1))
                    at_sb = absb.tile([128, igw], F16, tag="at")
                    nc.scalar.activation(at_sb[:], pa[:], AF.Copy)
                    nc.sync.dma_start(at_dram[js, ig], at_sb[:])

        # ---------- GCN layers ----------
        with (
            tc.tile_pool(name="aggps", bufs=2, space="PSUM") as aggps,
            tc.tile_pool(name="mmps", bufs=2, space="PSUM") as mmps,
            tc.tile_pool(name="tps", bufs=2, space="PSUM") as tps,
            tc.tile_pool(name="apool", bufs=6) as apool,
            tc.tile_pool(name="wpool", bufs=2) as wpool,
            tc.tile_pool(name="xpool", bufs=2) as xpool,
            tc.tile_pool(name="tpool", bufs=3) as tpool,
        ):
            for l in range(n_layers):
                beta = betas[l]
                cw1_sb = wpool.tile([128, 2, H], F16, tag="cw1")
                nc.sync.dma_start(cw1_sb[:], cw1_d.ap()[l])
                cw2_sb = wpool.tile([128, 2, H], F16, tag="cw2")
                nc.sync.dma_start(cw2_sb[:], cw2_d.ap()[l])
                hT_new = hT_pool.tile([128, 2, blk], F32, tag="hT")
                for ig in range(n_ig):
                    pa0 = aggps.tile([128, igw], F32, tag="agg0")
                    pa1 = aggps.tile([128, igw], F32, tag="agg1")
                    for js in range(n_js):
                        a_t = apool.tile([128, igw], F16, tag="a")
                        nc.sync.dma_start(a_t[:], at_dram[js, ig])
                        jc, jb = divmod(js, n_it)
                        nc.tensor.matmul(pa0[:], h16_full[:, jc, jb, 0:128], a_t[:],
                                         start=(js == 0), stop=(js == n_js - 1))
                        nc.tensor.matmul(pa1[:], h16_full[:, jc, jb, 128:256], a_t[:],
                                         start=(js == 0), stop=(js == n_js - 1))
                    xsT = xpool.tile([128, 2, igw], F32, tag="xsT")
                    nc.scalar.activation(xsT[:, 0, :], pa0[:], AF.Copy, scale=0.5)
                    nc.scalar.activation(xsT[:, 1, :], pa1[:], AF.Copy, scale=0.5)
                    xsT16 = xpool.tile([128, 2, igw], F16, tag="xsT16")
                    nc.vector.tensor_copy(xsT16[:], xsT[:])
                    sl = slice(ig * igw, (ig + 1) * igw)
                    for dh in range(2):
                        pmm = mmps.tile([128, igw], F32, tag="pmm")
                        nc.tensor.matmul(pmm[:], cw1_sb[:, 0, dh * 128:(dh + 1) * 128],
                                         xsT16[:, 0, :], start=True, stop=False)
                        nc.tensor.matmul(pmm[:], cw1_sb[:, 1, dh * 128:(dh + 1) * 128],
                                         xsT16[:, 1, :], start=False, stop=False)
                        nc.tensor.matmul(pmm[:], cw2_sb[:, 0, dh * 128:(dh + 1) * 128],
                                         x0sT16[:, 0, sl], start=False, stop=False)
                        nc.tensor.matmul(pmm[:], cw2_sb[:, 1, dh * 128:(dh + 1) * 128],
                                         x0sT16[:, 1, sl], start=False, stop=True)
                        t1 = tpool.tile([128, igw], F32, tag="t1")
                        nc.vector.tensor_add(t1[:], xsT[:, dh, :], x0sT[:, dh, sl])
                        t2 = tpool.tile([128, igw], F32, tag="t2")
                        nc.scalar.activation(t2[:], pmm[:], AF.Copy, scale=beta)
                        nc.vector.tensor_scalar_mul(t1[:], t1[:], 1.0 - beta)
                        nc.vector.tensor_add(t1[:], t1[:], t2[:])
                        nc.vector.tensor_add(t1[:], t1[:], hT[:, dh, sl])
                        nc.scalar.activation(hT_new[:, dh, sl], t1[:], AF.Relu)
                hT = hT_new
                if l < n_layers - 1:
                    h16_new = h16b_pool.tile([128, n_it, H], F16, tag="h16b")
                    for it in range(n_it):
                        for dh in range(2):
                            pt = tps.tile([128, 128], F32, tag="pt")
                            nc.tensor.transpose(
                                pt[:], hT[:, dh, it * 128:(it + 1) * 128], ident[:])
                            nc.scalar.activation(
                                h16_new[:, it, dh * 128:(dh + 1) * 128], pt[:], AF.Copy)
                    h16_full = allgather_h16(h16_new, "ag")
        embT = hT  # [128, 2, blk] f32
        G.close()  # release GCN-phase SBUF (h16_full, x0sT, h16_blk)
        spool = S.enter_context(tc.tile_pool(name="spool", bufs=1))

        # ---------- p_lc ----------
        with (
            tc.tile_pool(name="lcps", bufs=2, space="PSUM") as lcps,
            tc.tile_pool(name="lcsb", bufs=2) as lcsb,
        ):
            for it in range(n_it):
                plc = lcps.tile([128, C], F32, tag="plc")
                nc.tensor.matmul(plc[:], embT[:, 0, it * 128:(it + 1) * 128],
                                 w_out_sb[:, 0, :], start=True, stop=False)
                nc.tensor.matmul(plc[:], embT[:, 1, it * 128:(it + 1) * 128],
                                 w_out_sb[:, 1, :], start=False, stop=False)
                nc.tensor.matmul(plc[:], ones1[:], b_out_sb[:], start=False, stop=True)
                logsoftmax_from_psum(out_acc[:, it, :], plc, lcsb)

        # ---------- normalize ----------
        enT16_blk = spool.tile([128, 2, blk], F16)
        with (
            tc.tile_pool(name="nps", bufs=2, space="PSUM") as nps,
            tc.tile_pool(name="nsb", bufs=3) as nsb,
        ):
            en16_blk = nsb.tile([128, n_it, H], F16, tag="en16b", bufs=1)
            for it in range(n_it):
                pn0 = nps.tile([128, 128], F32, tag="pn0")
                nc.tensor.transpose(pn0[:], embT[:, 0, it * 128:(it + 1) * 128], ident[:])
                pn1 = nps.tile([128, 128], F32, tag="pn1")
                nc.tensor.transpose(pn1[:], embT[:, 1, it * 128:(it + 1) * 128], ident[:])
                emb_n = nsb.tile([128, H], F32, tag="embn")
                nc.scalar.activation(emb_n[:, 0:128], pn0[:], AF.Copy)
                nc.scalar.activation(emb_n[:, 128:256], pn1[:], AF.Copy)
                sq = nsb.tile([128, H], F32, tag="sq")
                ss = nsb.tile([128, 1], F32, tag="ss")
                nc.scalar.activation(sq[:], emb_n[:], AF.Square, accum_out=ss[:])
                nrm = nsb.tile([128, 1], F32, tag="nrm")
                nc.scalar.activation(nrm[:], ss[:], AF.Sqrt)
                nc.vector.tensor_scalar_max(nrm[:], nrm[:], 1e-8)
                inv = nsb.tile([128, 1], F32, tag="inv")
                nc.vector.reciprocal(inv[:], nrm[:])
                nc.vector.tensor_scalar(en16_blk[:, it, :], emb_n[:], inv[:], None,
                                        op0=ALU.mult)
                for dh in range(2):
                    pt = nps.tile([128, 128], F16, tag="pt2")
                    nc.tensor.transpose(
                        pt[:], en16_blk[:, it, dh * 128:(dh + 1) * 128], ident16[:])
                    nc.scalar.activation(
                        enT16_blk[:, dh, it * 128:(it + 1) * 128], pt[:], AF.Copy)
            gin2 = dram.tile([128, 2, blk], F16, tag="eg_in")
            nc.sync.dma_start(gin2[:], enT16_blk[:])
            gout2 = dram.tile([N_CORES, 128, 2, blk], F16, tag="eg_out",
                              addr_space="Shared")
            nc.gpsimd.collective_compute(
                "AllGather", ALU.bypass, replica_groups=groups,
                ins=[gin2[:].opt()], outs=[gout2[:].opt()])
            enT16_full = spool.tile([128, 2, N_CORES, blk], F16)
            nc.sync.dma_start(enT16_full[:], gout2[:].rearrange("c p h i -> p h c i"))

        # ---------- S1: per-row top-16 threshold tau ----------
        tau_rep = spool.tile([128, blk], F32)
        with (
            tc.tile_pool(name="sps", bufs=2, space="PSUM") as sps,
            tc.tile_pool(name="t8ps", bufs=2, space="PSUM") as t8ps,
            tc.tile_pool(name="s1sb", bufs=2) as s1sb,
        ):
            tau_col = s1sb.tile([128, n_it], F32, tag="tau_col", bufs=1)
            for it in range(n_it):
                cands = s1sb.tile([128, n_chunk * 16], F32, tag="cands")
                for ch in range(n_chunk):
                    strip = sps.tile([128, chunkw], F32, tag="strip")
                    for st in range(chunkw // subw):
                        j0 = ch * chunkw + st * subw
                        cb, off = divmod(j0, blk)
                        nc.tensor.matmul(
                            strip[:, st * subw:(st + 1) * subw],
                            enT16_blk[:, 0, it * 128:(it + 1) * 128],
                            enT16_full[:, 0, cb, off:off + subw],
                            start=True, stop=False)
                        nc.tensor.matmul(
                            strip[:, st * subw:(st + 1) * subw],
                            enT16_blk[:, 1, it * 128:(it + 1) * 128],
                            enT16_full[:, 1, cb, off:off + subw],
                            start=False, stop=True)
                    nc.vector.max(out=cands[:, ch * 16:ch * 16 + 8], in_=strip[:])
                    nc.vector.match_replace(out=strip[:],
                                            in_to_replace=cands[:, ch * 16:ch * 16 + 8],
                                            in_values=strip[:], imm_value=NEG)
                    nc.vector.max(out=cands[:, ch * 16 + 8:ch * 16 + 16], in_=strip[:])
                m1 = s1sb.tile([128, 8], F32, tag="m1")
                nc.vector.max(out=m1[:], in_=cands[:])
                nc.vector.match_replace(out=cands[:], in_to_replace=m1[:],
                                        in_values=cands[:], imm_value=NEG)
                m2 = s1sb.tile([128, 8], F32, tag="m2")
                nc.vector.max(out=m2[:], in_=cands[:])
                nc.vector.tensor_copy(tau_col[:, it:it + 1], m2[:, 7:8])
            # tau_col [128, n_it] -> tauT [n_it, 128] -> row [1, blk] -> tau_rep
            ptt = t8ps.tile([128, 128], F32, tag="ptt")
            nc.tensor.transpose(ptt[:n_it, :], tau_col[:], ident[:])
            tauT = s1sb.tile([n_it, 128], F32, tag="tauT", bufs=1)
            nc.scalar.activation(tauT[:], ptt[:n_it, :], AF.Copy)
            taurow = s1sb.tile([1, blk], F32, tag="taurow", bufs=1)
            nc.sync.dma_start(taurow[:], tauT[:])
            bw = min(512, blk)
            for bb in range(blk // bw):
                pb = t8ps.tile([128, bw], F32, tag="pb")
                nc.tensor.matmul(pb[:], ones1[:], taurow[:, bb * bw:(bb + 1) * bw],
                                 start=True, stop=True)
                nc.scalar.activation(tau_rep[:, bb * bw:(bb + 1) * bw], pb[:], AF.Copy)

        # ---------- S2: fused = (exp(sim) * (sim >= tau)) @ OH; p_sim ----------
        with (
            tc.tile_pool(name="simps", bufs=3, space="PSUM") as simps,
            tc.tile_pool(name="fps", bufs=2, space="PSUM") as fps,
            tc.tile_pool(name="ftps", bufs=2, space="PSUM") as ftps,
            tc.tile_pool(name="s2sb", bufs=3) as s2sb,
        ):
            for ig in range(n_ig):
                sl = slice(ig * igw, (ig + 1) * igw)
                pfused = fps.tile([C, igw], F32, tag="pf")
                for jt in range(n_js):
                    cb, off = divmod(jt * 128, blk)
                    psim = simps.tile([128, igw], F32, tag="psim")
                    nc.tensor.matmul(psim[:], enT16_full[:, 0, cb, off:off + 128],
                                     enT16_blk[:, 0, sl], start=True, stop=False)
                    nc.tensor.matmul(psim[:], enT16_full[:, 1, cb, off:off + 128],
                                     enT16_blk[:, 1, sl], start=False, stop=True)
                    e16 = s2sb.tile([128, igw], F16, tag="e16")
                    nc.scalar.activation(e16[:], psim[:], AF.Exp)
                    mk16 = s2sb.tile([128, igw], F16, tag="mk16")
                    nc.vector.tensor_tensor(mk16[:], psim[:], tau_rep[:, sl],
                                            op=ALU.is_ge)
                    ew16 = s2sb.tile([128, igw], F16, tag="ew16")
                    nc.vector.tensor_mul(ew16[:], e16[:], mk16[:])
                    nc.tensor.matmul(pfused[:], oh_sb[:, jt, :], ew16[:],
                                     start=(jt == 0), stop=(jt == n_js - 1))
                fsb = s2sb.tile([C, igw], F32, tag="fsb")
                nc.scalar.activation(fsb[:], pfused[:], AF.Copy)
                for t in range(igw // 128):
                    it = ig * (igw // 128) + t
                    pft = ftps.tile([128, C], F32, tag="pft")
                    nc.tensor.transpose(pft[:, :C], fsb[:, t * 128:(t + 1) * 128],
                                        ident[:C, :C])
                    logsoftmax_from_psum(out_acc[:, it, :], pft[:, :C], s2sb,
                                         add_into=out_acc[:, it, :])
            nc.sync.dma_start(out_d.ap().rearrange("(it p) c -> p it c", p=128),
                              out_acc[:])

    nc.compile()
    return nc


def prep_inputs(inputs, n=N, n_layers=N_LAYERS):
    """Host-side sharding/layout prep. Returns in_maps (one dict per core)."""
    blk = n // N_CORES
    igw = min(512, blk)
    n_ig = blk // igw
    n_js = n // 128
    n_bkt = n_js * n_ig
    x = np.asarray(inputs["x"], np.float32)
    y = np.asarray(inputs["y"]).astype(np.int64)
    ei = np.asarray(inputs["edge_index"]).astype(np.int64)
    ew = np.asarray(inputs["edge_weight"], np.float32)
    src, dst = ei[0], ei[1]

    y32 = np.ascontiguousarray(
        y.reshape(n_js, 128).T.astype(np.float32))  # [128, n_js]
    w_in16 = np.ascontiguousarray(
        np.asarray(inputs["W_in"], np.float32)
        .reshape(D_IN // 128, 128, H).transpose(1, 0, 2)).astype(np.float16)
    b_in16 = np.asarray(inputs["b_in"], np.float32).reshape(1, H).astype(np.float16)
    w_out = np.ascontiguousarray(
        np.asarray(inputs["W_out"], np.float32)
        .reshape(2, 128, C).transpose(1, 0, 2))
    b_out = np.asarray(inputs["b_out"], np.float32).reshape(1, C)
    cw116 = np.ascontiguousarray(
        np.asarray(inputs["conv_w1"], np.float32)
        .reshape(n_layers, 2, 128, H).transpose(0, 2, 1, 3)).astype(np.float16)
    cw216 = np.ascontiguousarray(
        np.asarray(inputs["conv_w2"], np.float32)
        .reshape(n_layers, 2, 128, H).transpose(0, 2, 1, 3)).astype(np.float16)

    core_of = dst // blk
    in_maps = []
    for c in range(N_CORES):
        sel = core_of == c
        s, d, w = src[sel], dst[sel] - c * blk, ew[sel]
        js, sl_ = s >> 7, s & 127
        ig, dw = d // igw, d % igw
        bkt = js * n_ig + ig
        order = np.argsort(bkt, kind="stable")
        bs = bkt[order]
        counts = np.bincount(bs, minlength=n_bkt)
        if counts.max() > CAP:
            raise RuntimeError(f"edge bucket overflow: {counts.max()} > {CAP}")
        starts = np.zeros(n_bkt + 1, np.int64)
        np.cumsum(counts, out=starts[1:])
        pos = np.arange(len(bs)) - starts[bs]
        esrc_a = np.full((n_bkt, CAP), -1.0, np.float32)
        edst_a = np.full((n_bkt, CAP), -1.0, np.float32)
        ew_a = np.zeros((n_bkt, CAP), np.float32)
        esrc_a[bs, pos] = sl_[order]
        edst_a[bs, pos] = dw[order]
        ew_a[bs, pos] = w[order]
        # [n_js, n_ig, CAP] -> [128, n_js, n_ig, N_CH] with entry = ch*128 + p
        def to_dev(a):
            return np.ascontiguousarray(
                a.reshape(n_js, n_ig, N_CH, 128).transpose(3, 0, 1, 2))
        lo, hi = c * blk, (c + 1) * blk
        xT16 = np.ascontiguousarray(
            x[lo:hi].T.reshape(D_IN // 128, 128, blk)
            .transpose(1, 0, 2)).astype(np.float16)
        in_maps.append({
            "xT16": xT16, "esrc": to_dev(esrc_a), "edst": to_dev(edst_a),
            "ew": to_dev(ew_a), "y_r": y32, "w_in16": w_in16, "b_in16": b_in16,
            "cw116": cw116, "cw216": cw216, "w_out_r": w_out, "b_out_r": b_out,
        })
    return in_maps


_CACHED_NC = None


def kernel(**inputs):
    global _CACHED_NC
    if _CACHED_NC is None:
        _CACHED_NC = build_program()
    in_maps = prep_inputs(inputs)
    res = run_bass_kernel_spmd(_CACHED_NC, in_maps, core_ids=list(range(N_CORES)))
    out = np.concatenate([res.results[c]["out"] for c in range(N_CORES)], axis=0)
    return out.astype(np.float32)


if __name__ == "__main__":
    nc = build_program()
    print("built + compiled OK")
